# revision 5
# baseline (speedup 1.0000x reference)
"""GPT2 causal self-attention forward on 8 Trainium2 NeuronCores (Bass/Tile).

Contract: kernel(**inputs) takes the FULL inputs of reference.setup_inputs()
  hidden_states [4, 2048, 1024] f32, w_attn [1024, 3072] f32,
  b_attn [3072] f32, w_proj [1024, 1024] f32, b_proj [1024] f32
and returns the FULL output [4, 2048, 1024] f32.

Sharding (batch x head-half, no collectives): core c = 2*b + m handles batch b
and heads m*8..m*8+7.  Each core computes q/k/v for its 8 heads (columns of
w_attn), causal attention, and a partial output projection against its 512
rows of w_proj.  The host sums the two partial projections per batch and adds
b_proj (exact, since the partials split the contraction dimension).

Per-core device pipeline (all matmuls bf16 with f32 PSUM accumulation):
  hidT [D,S] (host-pre-transposed, bf16) -> qT/kT [c,S] and V [S,c] tiles;
  scoresT[k,q] per 128-k-tile with the two 64-dim heads of a pair row-tiled
  onto PE array halves (tile_position (0,0)/(64,0), concurrent on HW); one
  fused exp per pair on the scalar engine reading a 2-bank PSUM tile;
  causality via column slicing on diagonal tiles + one fused triangular
  band multiply; PV with a ones-augmented V so PSUM row 64 carries the
  softmax denominator; normalize via DVE reciprocal + gpsimd
  partition_broadcast + DVE multiply; projection accumulates 4 channel tiles.

Scheduling: QKV row-chunk 0 leads; remaining QKV chunks and all output
projections are emitted as filler between attention k-tile steps so the
tensor engine stays busy while the scalar engine works through the exps.
"""

import sys

for _p in ("/opt/trn_rl_repo", "/root/.axon_site/_ro/trn_rl_repo"):
    if _p not in sys.path:
        sys.path.append(_p)

# diagnostic phase gate: 0=DMA only, 1=+QKV, 2=+attention, 3=full (default)
# 4=QKV only with input DMA hoisted out of the loop, 5=DMA + minimal consumer
PHASE = 3

from collections import deque
from contextlib import ExitStack

import numpy as np
import ml_dtypes

import concourse.mybir as mybir
import concourse.tile as tile
from concourse import bacc
from concourse.bass_utils import run_bass_kernel_spmd

# The kernel's only transcendentals are Exp (softmax) and Ln (softmax
# denominator reciprocal via exp(-ln z)).  The stock table chooser maps Exp
# to exp_and_others and Ln to natural_log, thrashing ACT_TABLE_LOAD (2.6us)
# at every head-pair boundary — 33 loads / 42us per run measured.  Strip Exp
# and Ln from every other set so both resolve to natural_log_exp_and_others
# (set order, and hence act_func_set ids, are preserved).
_ORIG_ACT_TABLES = bacc.get_activation_tables


def _act_tables_single_exp_ln_set(arch):
    tables = _ORIG_ACT_TABLES(arch)
    aft = mybir.ActivationFunctionType
    want = "natural_log_exp_and_others"
    if want not in tables:
        return tables
    return {name: (fns if name == want else fns - {aft.Exp, aft.Ln})
            for name, fns in tables.items()}


bacc.get_activation_tables = _act_tables_single_exp_ln_set

FP32 = mybir.dt.float32
BF16 = mybir.dt.bfloat16
AF = mybir.ActivationFunctionType
OP = mybir.AluOpType

S, D, H, HD = 2048, 1024, 8, 64   # per-core: sequence, model dim, heads, head dim
P = 128
C = H * HD                         # 512 channels per core
KD = D // P                        # 8 contraction tiles over D
RC, QB = 512, 512                  # row chunk / query block
NRT, NRC, NQB = S // P, S // RC, S // QB
NCT = C // P                       # 4 channel tiles (2 heads each)
NPJ = D // 512                     # 2 projection column chunks

_NC_CACHE = {}


def _build_kernel(iters=1):
    nc = bacc.Bacc("TRN2", target_bir_lowering=False, debug=False, num_devices=8)

    hidT_d = nc.dram_tensor("hidT", [D, S], BF16, kind="ExternalInput")
    wq_d = nc.dram_tensor("wq", [D, C], BF16, kind="ExternalInput")
    wk_d = nc.dram_tensor("wk", [D, C], BF16, kind="ExternalInput")
    wv_d = nc.dram_tensor("wv", [D, C], BF16, kind="ExternalInput")
    bq_d = nc.dram_tensor("bq", [C], FP32, kind="ExternalInput")
    bk_d = nc.dram_tensor("bk", [C], FP32, kind="ExternalInput")
    bv_d = nc.dram_tensor("bv", [C], FP32, kind="ExternalInput")
    wp_d = nc.dram_tensor("wp", [C, D], BF16, kind="ExternalInput")
    out_d = nc.dram_tensor("out", [S, D], FP32, kind="ExternalOutput")

    with tile.TileContext(nc) as tc:
        with (
            tc.tile_pool(name="persist", bufs=1) as pp,
            tc.tile_pool(name="pt_pool", bufs=8) as ptp,
            tc.tile_pool(name="work", bufs=3) as wkp,
            tc.tile_pool(name="outp", bufs=3) as obp,
            tc.tile_pool(name="mm", bufs=2, space="PSUM") as mmp,
            tc.tile_pool(name="sc", bufs=2, space="PSUM") as scp,
            tc.tile_pool(name="pv", bufs=1, space="PSUM") as pvp,
        ):
            tiles = _alloc_and_static(nc, pp)
            if PHASE == 4:
                _emit_dma(nc, tiles, hidT_d, wq_d, wk_d, wv_d,
                          bq_d, bk_d, bv_d, wp_d)
            with ExitStack() as loop_ctx:
                if iters > 1:
                    # body is >256 instructions on PE/DVE/ACT: arm the
                    # back-edge branch prefetch so the jump I$-hits
                    loop_ctx.enter_context(tc.For_i(
                        0, iters,
                        hint_engines=(mybir.EngineType.PE,
                                      mybir.EngineType.DVE,
                                      mybir.EngineType.Activation)))
                _emit_body(nc, tc, tiles, ptp, wkp, obp, mmp, scp, pvp,
                           hidT_d, wq_d, wk_d, wv_d, bq_d, bk_d, bv_d,
                           wp_d, out_d)

    nc.compile()
    return nc


def _alloc_and_static(nc, pp):
    """Persistent tiles + input-independent setup (outside the timing loop)."""
    t = {}
    t["hidT"] = pp.tile([P, KD, S], BF16, tag="hidT", name="hidT_sb")
    t["wq"] = pp.tile([P, KD, C], BF16, tag="wq", name="wq_sb")
    t["wk"] = pp.tile([P, KD, C], BF16, tag="wk", name="wk_sb")
    t["wv"] = pp.tile([P, KD, C], BF16, tag="wv", name="wv_sb")
    t["wp"] = pp.tile([P, NCT, D], BF16, tag="wp", name="wp_sb")
    t["bq"] = pp.tile([P, NCT], FP32, tag="bq", name="bq_sb")
    t["bk"] = pp.tile([P, NCT], FP32, tag="bk", name="bk_sb")
    t["bv1"] = pp.tile([1, C], FP32, tag="bv1", name="bv1_sb")
    t["bvb"] = pp.tile([P, H, HD], FP32, tag="bvb", name="bvb_sb")
    maskf = pp.tile([P, P], FP32, tag="maskf", name="maskf_sb")
    maskb2 = pp.tile([P, 2, P - 1], BF16, tag="maskb2", name="maskb2_sb")
    t["maskb2"] = maskb2
    t["qT"] = [pp.tile([P, S], BF16, tag=f"qT{ct}", name=f"qT{ct}_sb")
               for ct in range(NCT)]
    t["kT"] = [pp.tile([P, S], BF16, tag=f"kT{ct}", name=f"kT{ct}_sb")
               for ct in range(NCT)]
    t["vaug"] = [pp.tile([P, H, HD + 1], BF16, tag=f"va{rt}", name=f"va{rt}_sb")
                 for rt in range(NRT)]
    t["aT"] = [pp.tile([P, S], BF16, tag=f"aT{ct}", name=f"aT{ct}_sb")
               for ct in range(NCT)]

    # triangular mask band, duplicated per head: maskb2[kk, i, u] = u >= kk
    nc.vector.memset(maskf[:], 1.0)
    nc.gpsimd.affine_select(
        out=maskf[:], in_=maskf[:], compare_op=OP.is_ge, fill=0.0,
        base=0, channel_multiplier=-1, pattern=[[1, P]])
    for i in range(2):
        nc.vector.tensor_copy(maskb2[:, i, :], maskf[:, 0:P - 1])
    # the ones column of the augmented V never changes
    for rt in range(NRT):
        nc.vector.memset(t["vaug"][rt][:, :, HD:HD + 1], 1.0)
    return t


def _emit_dma(nc, t, hidT_d, wq_d, wk_d, wv_d, bq_d, bk_d, bv_d, wp_d):
    # ---- input DMA, ordered by first use: biases, then the k path for
    # row-chunk 0 (hidT split per row chunk), q, v, remaining hidT, wp ----
    hidT, wq, wk, wv, wp = t["hidT"], t["wq"], t["wk"], t["wv"], t["wp"]
    bq, bk, bv1 = t["bq"], t["bk"], t["bv1"]
    hidT_r = hidT_d.rearrange("(j p) s -> p j s", p=P)
    nc.sync.dma_start(bk[:], bk_d.rearrange("(ct p) -> p ct", p=P))
    nc.sync.dma_start(bq[:], bq_d.rearrange("(ct p) -> p ct", p=P))
    nc.sync.dma_start(bv1[:], bv_d[None, :])
    # one ~1MB DMA per tensor / row chunk (descriptors are 1-4KB per
    # partition row), ordered by first use: k path, q, v, late hidT, wp
    nc.sync.dma_start(wk[:], wk_d.rearrange("(j p) c -> p j c", p=P))
    nc.sync.dma_start(hidT[:, :, 0:RC], hidT_r[:, :, 0:RC])
    nc.sync.dma_start(wq[:], wq_d.rearrange("(j p) c -> p j c", p=P))
    nc.sync.dma_start(wv[:], wv_d.rearrange("(j p) c -> p j c", p=P))
    for rc in range(1, NRC):
        cols = slice(rc * RC, (rc + 1) * RC)
        nc.sync.dma_start(hidT[:, :, cols], hidT_r[:, :, cols])
    nc.sync.dma_start(wp[:], wp_d.rearrange("(ct p) d -> p ct d", p=P))


def _emit_body(nc, tc, t, ptp, wkp, obp, mmp, scp, pvp,
               hidT_d, wq_d, wk_d, wv_d, bq_d, bk_d, bv_d, wp_d, out_d):
    hidT, wq, wk, wv, wp = t["hidT"], t["wq"], t["wk"], t["wv"], t["wp"]
    bq, bk, bv1, bvb = t["bq"], t["bk"], t["bv1"], t["bvb"]
    maskb2, qT, kT, vaug, aT = t["maskb2"], t["qT"], t["kT"], t["vaug"], t["aT"]

    if PHASE != 4:
        _emit_dma(nc, t, hidT_d, wq_d, wk_d, wv_d, bq_d, bk_d, bv_d, wp_d)

    # broadcast bv across partitions: bvb[p, h, dd] = bv[64h+dd]
    nc.gpsimd.partition_broadcast(
        bvb.rearrange("p h d -> p (h d)"), bv1[:], channels=P)

    # ---- QKV / projection op groups ----
    def g_qk(rc, ct, wt, bt, dst):
        cols = slice(rc * RC, (rc + 1) * RC)
        ccols = slice(ct * P, (ct + 1) * P)
        ps = mmp.tile([P, RC], FP32, tag="mm", name="qk_ps")
        for j in range(KD):
            nc.tensor.matmul(
                ps[:], wt[:, j, ccols], hidT[:, j, cols],
                start=(j == 0), stop=(j == KD - 1))
        nc.vector.tensor_scalar_add(dst[ct][:, cols], ps[:], bt[:, ct:ct + 1])

    def g_v(rt):
        rows = slice(rt * P, (rt + 1) * P)
        ps = mmp.tile([P, C], FP32, tag="mm", name="v_ps")
        for j in range(KD):
            nc.tensor.matmul(
                ps[:], hidT[:, j, rows], wv[:, j, :],
                start=(j == 0), stop=(j == KD - 1))
        nc.vector.tensor_tensor(
            vaug[rt][:, :, 0:HD],
            ps.rearrange("p (h d) -> p h d", h=H), bvb[:], OP.add)

    def g_proj(rt, nj):
        rows = slice(rt * P, (rt + 1) * P)
        ncols = slice(nj * 512, (nj + 1) * 512)
        ps = mmp.tile([P, 512], FP32, tag="mm", name="pj_ps")
        for ct in range(NCT):
            nc.tensor.matmul(
                ps[:], aT[ct][:, rows], wp[:, ct, ncols],
                start=(ct == 0), stop=(ct == NCT - 1))
        ob = obp.tile([P, 512], FP32, tag="ob", name="ob_t")
        nc.vector.tensor_copy(ob[:], ps[:])
        nc.sync.dma_start(out_d[rows, ncols], ob[:])

    def dummy_out():
        ob = obp.tile([P, 512], FP32, tag="ob", name="ob_t")
        nc.vector.memset(ob[:], 0.0)
        nc.sync.dma_start(out_d[0:P, 0:512], ob[:])

    if PHASE == 0:
        dummy_out()
        return

    if PHASE == 5:
        # minimal consumers that force a wait on every input DMA
        srcs = [(wk, 0), (wq, 1), (wv, 2), (wk, 3)]
        for rc in range(NRC):
            wt, _ = srcs[rc]
            ps = mmp.tile([P, RC], FP32, tag="mm", name="d_ps")
            cols = slice(rc * RC, (rc + 1) * RC)
            for j in range(KD):
                nc.tensor.matmul(ps[:], wt[:, j, 0:P], hidT[:, j, cols],
                                 start=(j == 0), stop=(j == KD - 1))
            nc.vector.tensor_scalar_add(qT[0][:, cols], ps[:], bq[:, 0:1])
        psw = mmp.tile([P, RC], FP32, tag="mm", name="d_psw")
        for ct in range(NCT):
            nc.tensor.matmul(psw[:], wp[:, ct, 0:P], hidT[:, 0, 0:RC],
                             start=(ct == 0), stop=(ct == NCT - 1))
        nc.vector.tensor_scalar_add(kT[0][:, 0:RC], psw[:], bk[:, 0:1])
        dummy_out()
        return

    # eager lead-in: rc0 k/q for ct0 plus all of v rows 0..511
    g_qk(0, 0, wk, bk, kT)
    g_qk(0, 0, wq, bq, qT)
    for rt in range(4):
        g_v(rt)

    # filler queue (emitted between attention k-tile steps)
    filler = deque()
    for ct in range(1, NCT):
        filler.append(lambda ct=ct: g_qk(0, ct, wk, bk, kT))
        filler.append(lambda ct=ct: g_qk(0, ct, wq, bq, qT))
    for rc in range(1, NRC):
        filler.append(lambda rc=rc: g_qk(rc, 0, wk, bk, kT))
        filler.append(lambda rc=rc: g_qk(rc, 0, wq, bq, qT))
        for rt in range(rc * 4, rc * 4 + 4):
            filler.append(lambda rt=rt: g_v(rt))
        for ct in range(1, NCT):
            filler.append(lambda rc=rc, ct=ct: g_qk(rc, ct, wk, bk, kT))
            filler.append(lambda rc=rc, ct=ct: g_qk(rc, ct, wq, bq, qT))
    emitted = 0

    def fill(n):
        nonlocal emitted
        for _ in range(n):
            if not filler:
                return
            filler.popleft()()
            emitted += 1

    if PHASE == 1:
        fill(len(filler))
        dummy_out()
        return

    def drain_to(n):
        fill(max(0, n - emitted))

    def qkv_prefix(qb, hp):
        # groups that must be emitted before attn(qb, hp) starts
        if qb == 0:
            return 2 * hp
        return 6 + 12 * (qb - 1) + (6 + 2 * hp if hp else 6)

    # ---- attention: per q block x head pair, k-tiles pipelined ----
    PF = 3
    for qb in range(NQB):
        qcols = slice(qb * QB, (qb + 1) * QB)
        nkt = (qb + 1) * QB // P
        if qb == NQB - 1 and PHASE >= 3:
            # all QKV is drained by now; queue deferred projections
            drain_to(6 + 12 * 3)
            for pqb in range(NQB - 1):
                for rt in range(pqb * 4, pqb * 4 + 4):
                    for nj in range(NPJ):
                        filler.append(lambda rt=rt, nj=nj: g_proj(rt, nj))
        for hp in range(NCT):
            drain_to(qkv_prefix(qb, hp))
            pv = pvp.tile([HD + 1, 2, QB], FP32, tag="pv",
                          name=f"pv_{qb}_{hp}")

            def probs(j, qb=qb, hp=hp):
                """row-tiled scores pair + fused exp (+ causal band)."""
                kcols = slice(j * P, (j + 1) * P)
                o = (j - qb * QB // P) * P  # >=0 on diagonal tiles
                qs = slice(qb * QB + max(o, 0), (qb + 1) * QB)
                w = QB - max(o, 0)
                sc2 = scp.tile([P, 2, QB], FP32, tag="sc",
                               name=f"sc_{qb}_{hp}_{j}")
                for i in range(2):
                    hrows = slice(i * HD, (i + 1) * HD)
                    nc.tensor.matmul(sc2[:, i, :w], kT[hp][hrows, kcols],
                                     qT[hp][hrows, qs],
                                     start=True, stop=True,
                                     tile_position=(i * HD, 0))
                pt2 = ptp.tile([P, 2, QB], BF16, tag="pt", name="pt_t")
                nc.scalar.activation(pt2[:, :, :w], sc2[:, :, :w], AF.Exp,
                                     scale=float(HD) ** -0.5)
                if o >= 0:
                    nc.vector.tensor_tensor(
                        pt2[:, :, 0:P - 1], pt2[:, :, 0:P - 1],
                        maskb2[:], OP.mult)
                return pt2, w

            pending = [probs(j) for j in range(min(PF, nkt))]
            for j in range(nkt):
                pt2, w = pending.pop(0)
                for i in range(2):
                    nc.tensor.matmul(
                        pv[:, i, QB - w:], vaug[j][:, 2 * hp + i, :],
                        pt2[:, i, :w],
                        start=(j == 0), stop=(j == nkt - 1),
                        skip_group_check=True)
                if j + PF < nkt:
                    fill(1)
                    pending.append(probs(j + PF))
            # spill pv to SBUF with one DVE copy so the PSUM banks free up
            # quickly; normalize (row 64 holds the denominator) off-path.
            # Last pair of the last q block: nothing reuses the banks, so
            # skip the spill to shorten the projection tail.
            last = (qb == NQB - 1 and hp == NCT - 1)
            if last:
                pvs = pv
            else:
                pvs = wkp.tile([HD + 1, 2, QB], BF16, tag="pvs", name="pvs_t")
                nc.vector.tensor_copy(pvs[:], pv[:])
            # 1/z via ACT ln + exp(-x): both live in the
            # natural_log_exp_and_others table set (no table reloads), and
            # ACT runs 1 elem/cycle vs DVE reciprocal's 8 — the [1, 1024]
            # single-partition reciprocal measured 6.5us per call on HW.
            lz = wkp.tile([1, 2, QB], FP32, tag="lz", name="lz_t")
            nc.scalar.activation(lz[:], pvs[HD:HD + 1, :, :], AF.Ln)
            rz = wkp.tile([1, 2, QB], BF16, tag="rz", name="rz_t")
            nc.scalar.activation(rz[:], lz[:], AF.Exp, scale=-1.0)
            bz = wkp.tile([HD, 2, QB], BF16, tag="bz", name="bz_t")
            nc.gpsimd.partition_broadcast(
                bz.rearrange("p i q -> p (i q)"),
                rz.rearrange("p i q -> p (i q)"), channels=HD)
            for i in range(2):
                nc.vector.tensor_tensor(
                    aT[hp][i * HD:(i + 1) * HD, qcols],
                    pvs[0:HD, i, :], bz[:, i, :], OP.mult)

    # tail: any remaining fillers, then the last q block's projection
    fill(len(filler))
    if PHASE < 3:
        dummy_out()
        return
    for rt in range((NQB - 1) * 4, NQB * 4):
        for nj in range(NPJ):
            g_proj(rt, nj)


def shard_inputs(hidden_states, w_attn, b_attn, w_proj):
    hidden_states = np.asarray(hidden_states, dtype=np.float32)
    w_attn = np.asarray(w_attn, dtype=np.float32)
    b_attn = np.asarray(b_attn, dtype=np.float32)
    w_proj = np.asarray(w_proj, dtype=np.float32)
    bf16 = ml_dtypes.bfloat16

    in_maps = []
    for c in range(8):
        b, m = divmod(c, 2)
        sl = slice(m * C, (m + 1) * C)
        in_maps.append(dict(
            hidT=np.ascontiguousarray(hidden_states[b].T).astype(bf16),
            wq=np.ascontiguousarray(w_attn[:, sl]).astype(bf16),
            wk=np.ascontiguousarray(w_attn[:, D + m * C:D + (m + 1) * C]).astype(bf16),
            wv=np.ascontiguousarray(w_attn[:, 2 * D + m * C:2 * D + (m + 1) * C]).astype(bf16),
            bq=np.ascontiguousarray(b_attn[sl]),
            bk=np.ascontiguousarray(b_attn[D + m * C:D + (m + 1) * C]),
            bv=np.ascontiguousarray(b_attn[2 * D + m * C:2 * D + (m + 1) * C]),
            wp=np.ascontiguousarray(w_proj[sl, :]).astype(bf16),
        ))
    return in_maps


def assemble_output(results, b_proj):
    b_proj = np.asarray(b_proj, dtype=np.float32)
    outs = [r["out"] for r in results]
    return np.stack([outs[2 * b] + outs[2 * b + 1] + b_proj[None, :]
                     for b in range(4)]).astype(np.float32)


def kernel(hidden_states, w_attn, b_attn, w_proj, b_proj):
    if "nc" not in _NC_CACHE:
        _NC_CACHE["nc"] = _build_kernel()
    nc = _NC_CACHE["nc"]

    in_maps = shard_inputs(hidden_states, w_attn, b_attn, w_proj)
    res = run_bass_kernel_spmd(nc, in_maps, core_ids=list(range(8)))
    return assemble_output(res.results, b_proj)


if __name__ == "__main__":
    rng = np.random.default_rng(0)
    hs = rng.standard_normal((4, S, D)).astype(np.float32)
    wa = (rng.standard_normal((D, 3 * D)) * 0.02).astype(np.float32)
    ba = np.zeros(3 * D, np.float32)
    wpj = (rng.standard_normal((D, D)) * 0.02).astype(np.float32)
    bpj = np.zeros(D, np.float32)
    out = kernel(hs, wa, ba, wpj, bpj)
    print("kernel out", out.shape, out.dtype, float(np.abs(out).max()))

